# revision 1
# baseline (speedup 1.0000x reference)
"""Tensor-parallel LlamaAttention (S=2048, HID=4096, NH=32, NKV=8) on 8 trn2 cores.

Sharding: core c owns q heads {c, c+8, c+16, c+24} (head h uses kv head h%8,
so all four share kv head c) and kv head c.  Projections + attention are fully
local; avT (bf16, [128d, 2048s] per head) is AllGathered per head-group, then
each core computes its 512 output columns of o_proj (column-parallel wo).

Self-contained: shapes/sharding hardcoded; host does transposes/casts.
"""

import numpy as np
import ml_dtypes

import concourse.bacc as bacc
import concourse.tile as tile
import concourse.mybir as mybir
from concourse.bass_utils import run_bass_kernel_spmd

S = 2048
HID = 4096
NH = 32
NKV = 8
HD = 128
HALF = 64
N_CORES = 8
NREP = NH // NKV  # 4 q heads per core
NHT = HID // 128  # 32 hidden tiles
NST = S // 128    # 16 seq tiles
NSC = S // 512    # 4 seq chunks
BF16 = mybir.dt.bfloat16
F32 = mybir.dt.float32

_CACHE = {}


def build_nc():
    nc = bacc.Bacc("TRN2", target_bir_lowering=False, debug=False,
                   num_devices=N_CORES)

    xT = nc.dram_tensor("xT", [HID, S], BF16, kind="ExternalInput").ap()
    wq = nc.dram_tensor("wqT", [HID, NREP * HD], BF16, kind="ExternalInput").ap()
    wk = nc.dram_tensor("wkT", [HID, HD], BF16, kind="ExternalInput").ap()
    wv = nc.dram_tensor("wvT", [HID, HD], BF16, kind="ExternalInput").ap()
    wo = nc.dram_tensor("woT", [HID, 512], BF16, kind="ExternalInput").ap()
    cosT = nc.dram_tensor("cosT", [HD, S], F32, kind="ExternalInput").ap()
    sinT = nc.dram_tensor("sinT", [HD, S], F32, kind="ExternalInput").ap()
    tri = nc.dram_tensor("triT", [128, 128], BF16, kind="ExternalInput").ap()
    ones_c = nc.dram_tensor("ones_c", [128, 1], BF16, kind="ExternalInput").ap()
    ones_r = nc.dram_tensor("ones_r", [1, 128], F32, kind="ExternalInput").ap()

    o_out = nc.dram_tensor("o_out", [S, 512], F32, kind="ExternalOutput").ap()

    ag_in = [nc.dram_tensor(f"ag_in{j}", [HD, S], BF16).ap() for j in range(NREP)]
    ag_out = [nc.dram_tensor(f"ag_out{j}", [N_CORES * HD, S], BF16,
                             addr_space="Shared").ap() for j in range(NREP)]

    with tile.TileContext(nc) as tc:
        _body(nc, tc, xT, wq, wk, wv, wo, cosT, sinT, tri, ones_c, ones_r,
              o_out, ag_in, ag_out)
    nc.compile()
    return nc


def _body(nc, tc, xT, wq, wk, wv, wo, cosT, sinT, tri, ones_c, ones_r,
          o_out, ag_in, ag_out):
    Exp = mybir.ActivationFunctionType.Exp
    with (
        tc.tile_pool(name="consts", bufs=1) as cpool,
        tc.tile_pool(name="psum", bufs=4, space="PSUM") as psum,
        tc.tile_pool(name="psav", bufs=2, space="PSUM") as psav,
        tc.tile_pool(name="psrs", bufs=2, space="PSUM") as psrs,
    ):
        # ---- small constants (live through phase 2) ----
        tri_sb = cpool.tile([128, 128], BF16, tag="tri")
        onc_sb = cpool.tile([128, 1], BF16, tag="onc")
        onr_sb = cpool.tile([1, 128], F32, tag="onr")
        nc.sync.dma_start(out=tri_sb[:], in_=tri[:])
        nc.sync.dma_start(out=onc_sb[:], in_=ones_c[:])
        nc.sync.dma_start(out=onr_sb[:], in_=ones_r[:])

        _phases12(nc, tc, xT, wq, wk, wv, cosT, sinT, tri_sb, onc_sb, onr_sb,
                  ag_in, ag_out, psum, psav, psrs)
        _phase3(nc, tc, wo, o_out, ag_out, psum)


def _phases12(nc, tc, xT, wq, wk, wv, cosT, sinT, tri_sb, onc_sb, onr_sb,
              ag_in, ag_out, psum, psav, psrs):
    Exp = mybir.ActivationFunctionType.Exp
    with tc.tile_pool(name="qkv", bufs=1) as qkvpool:
        # ---- projection outputs (resident, bf16) ----
        qT_sb = [qkvpool.tile([HD, S], BF16, tag=f"qT{j}", name=f"qT{j}")
                 for j in range(NREP)]
        kT_sb = qkvpool.tile([HD, S], BF16, tag="kT")
        v_sb = qkvpool.tile([128, S], BF16, tag="v")  # col block kt = s tile kt

        with (
            tc.tile_pool(name="rconsts", bufs=1) as rcpool,
            tc.tile_pool(name="wproj", bufs=1) as wpool,
            tc.tile_pool(name="xc", bufs=64) as xpool,
            tc.tile_pool(name="rope", bufs=2) as rpool,
        ):
            _phase1(nc, tc, xT, wq, wk, wv, cosT, sinT, qT_sb, kT_sb, v_sb,
                    rcpool, wpool, xpool, rpool, psum)
        with (
            tc.tile_pool(name="probs", bufs=16) as ppool,
            tc.tile_pool(name="avt", bufs=1) as avpool,
            tc.tile_pool(name="small", bufs=2) as spool,
        ):
            _phase2(nc, tc, qT_sb, kT_sb, v_sb, tri_sb, onc_sb, onr_sb,
                    ag_in, ag_out, ppool, avpool, spool, psum, psav, psrs)


def _phase1(nc, tc, xT, wq, wk, wv, cosT, sinT, qT_sb, kT_sb, v_sb,
            rcpool, wpool, xpool, rpool, psum):
    # ---- rope tables ----
    cos_sb = rcpool.tile([HD, S], F32, tag="cos")
    sin_sb = rcpool.tile([HD, S], F32, tag="sin")
    nc.sync.dma_start(out=cos_sb[:], in_=cosT[:])
    nc.sync.dma_start(out=sin_sb[:], in_=sinT[:])

    # ---- weights (resident; column block h = hidden tile h) ----
    wq_sb = wpool.tile([128, NHT * 512], BF16, tag="wq")
    wk_sb = wpool.tile([128, NHT * 128], BF16, tag="wk")
    wv_sb = wpool.tile([128, NHT * 128], BF16, tag="wv")
    for h in range(NHT):
        nc.sync.dma_start(out=wq_sb[:, h * 512:(h + 1) * 512],
                          in_=wq[h * 128:(h + 1) * 128, :])
        nc.sync.dma_start(out=wk_sb[:, h * 128:(h + 1) * 128],
                          in_=wk[h * 128:(h + 1) * 128, :])
        nc.sync.dma_start(out=wv_sb[:, h * 128:(h + 1) * 128],
                          in_=wv[h * 128:(h + 1) * 128, :])

    if True:
        for cs in range(NSC):
            sc = slice(cs * 512, (cs + 1) * 512)
            xcs = [xpool.tile([128, 512], BF16, tag="xc", name=f"xc{cs}_{h}")
                   for h in range(NHT)]
            for h in range(NHT):
                nc.sync.dma_start(out=xcs[h][:],
                                  in_=xT[h * 128:(h + 1) * 128, sc])

            def _rope(dst, pp):
                t1 = rpool.tile([HALF, 512], F32, tag="t1")
                t2 = rpool.tile([HALF, 512], F32, tag="t2")
                nc.vector.tensor_mul(t1[:], pp[0:HALF, :], cos_sb[0:HALF, sc])
                nc.vector.tensor_mul(t2[:], pp[HALF:128, :], sin_sb[0:HALF, sc])
                nc.vector.tensor_sub(dst[0:HALF, sc], t1[:], t2[:])
                t3 = rpool.tile([HALF, 512], F32, tag="t1")
                t4 = rpool.tile([HALF, 512], F32, tag="t2")
                nc.vector.tensor_mul(t3[:], pp[HALF:128, :], cos_sb[HALF:128, sc])
                nc.vector.tensor_mul(t4[:], pp[0:HALF, :], sin_sb[HALF:128, sc])
                nc.vector.tensor_add(dst[HALF:128, sc], t3[:], t4[:])

            for j in range(NREP):
                pq = psum.tile([128, 512], F32, tag="mm")
                for h in range(NHT):
                    nc.tensor.matmul(
                        pq[:],
                        wq_sb[:, h * 512 + j * 128: h * 512 + (j + 1) * 128],
                        xcs[h][:],
                        start=(h == 0), stop=(h == NHT - 1))
                _rope(qT_sb[j], pq)

            pk = psum.tile([128, 512], F32, tag="mm")
            for h in range(NHT):
                nc.tensor.matmul(pk[:], wk_sb[:, h * 128:(h + 1) * 128],
                                 xcs[h][:],
                                 start=(h == 0), stop=(h == NHT - 1))
            _rope(kT_sb, pk)

            pv = psum.tile([128, 512], F32, tag="mm")
            for tl in range(4):
                for h in range(NHT):
                    nc.tensor.matmul(
                        pv[:, tl * 128:(tl + 1) * 128],
                        xcs[h][:, tl * 128:(tl + 1) * 128],
                        wv_sb[:, h * 128:(h + 1) * 128],
                        start=(h == 0), stop=(h == NHT - 1))
            nc.scalar.copy(v_sb[:, sc], pv[:])



def _phase2(nc, tc, qT_sb, kT_sb, v_sb, tri_sb, onc_sb, onr_sb,
            ag_in, ag_out, ppool, avpool, spool, psum, psav, psrs):
    Exp = mybir.ActivationFunctionType.Exp
    # ---- phase 2: attention (scores transposed: [k, sq]) ----
    # Software-pipelined: PE does scores(kt+1..kt+3) while ACT exps kt, so the
    # rowsum/attnv matmuls for kt never stall on the exp.  Normalization of
    # chunk (j,C) is deferred into the next chunk's kt loop.
    avT_sb = [avpool.tile([HD, S], BF16, tag=f"av{j}", name=f"avT{j}")
              for j in range(NREP)]
    DEPTH = 3
    carry = [None]

    def make_norm(j, C, pav, prs):
        def f():
            qc = slice(C * 512, (C + 1) * 512)
            rrec = spool.tile([1, 512], F32, tag="rrec", name=f"rrec{j}_{C}")
            nc.vector.reciprocal(rrec[:], prs[:])
            pb = psum.tile([128, 512], F32, tag="mm", name=f"pb{j}_{C}")
            nc.tensor.matmul(pb[:], onr_sb[:], rrec[:], start=True, stop=True)
            bsb = spool.tile([128, 512], F32, tag="bsb", name=f"bsb{j}_{C}")
            nc.scalar.copy(bsb[:], pb[:])
            nc.vector.tensor_mul(avT_sb[j][:, qc], pav[:], bsb[:])
            if C == NSC - 1:
                nc.sync.dma_start(out=ag_in[j][:], in_=avT_sb[j][:])
                nc.gpsimd.collective_compute(
                    "AllGather", mybir.AluOpType.bypass,
                    replica_groups=[list(range(N_CORES))],
                    ins=[ag_in[j][:]], outs=[ag_out[j][:]])
        return f

    for j in range(NREP):
        for C in range(NSC):
            qc0 = C * 512
            nkt = 4 * C + 4
            pav = psav.tile([128, 512], F32, tag="av", name=f"pav{j}_{C}")
            prs = psrs.tile([1, 512], F32, tag="rs", name=f"prs{j}_{C}")
            pend = []

            def drain_one():
                kt2, pt2 = pend.pop(0)
                nc.tensor.matmul(prs[:], onc_sb[:], pt2[:],
                                 start=(kt2 == 0), stop=(kt2 == nkt - 1))
                nc.tensor.matmul(pav[:], v_sb[:, kt2 * 128:(kt2 + 1) * 128],
                                 pt2[:], start=(kt2 == 0), stop=(kt2 == nkt - 1))

            for kt in range(nkt):
                off = max(0, (kt - 4 * C) * 128)  # cols < off fully masked
                ps = psum.tile([128, 512], F32, tag="mm", name=f"ps{j}_{C}_{kt}")
                nc.tensor.matmul(ps[:, off:512],
                                 kT_sb[:, kt * 128:(kt + 1) * 128],
                                 qT_sb[j][:, qc0 + off: qc0 + 512],
                                 start=True, stop=True)
                pt = ppool.tile([128, 512], BF16, tag="pt", name=f"pt{j}_{C}_{kt}")
                if off > 0:
                    nc.vector.memset(pt[:, 0:off], 0.0)
                nc.scalar.activation(pt[:, off:512], ps[:, off:512], Exp)
                if kt >= 4 * C:
                    nc.vector.tensor_mul(pt[:, off:off + 128],
                                         pt[:, off:off + 128], tri_sb[:])
                pend.append((kt, pt))
                if kt == 1 and carry[0] is not None:
                    carry[0]()
                    carry[0] = None
                if len(pend) > DEPTH:
                    drain_one()
            while pend:
                drain_one()
            carry[0] = make_norm(j, C, pav, prs)
    carry[0]()
    carry[0] = None



def _phase3(nc, tc, wo, o_out, ag_out, psum):
    # ---- phase 3: column-parallel o_proj ----
    if True:
        with (
            tc.tile_pool(name="ag", bufs=1) as agpool,
            tc.tile_pool(name="wo", bufs=1) as wopool,
            tc.tile_pool(name="oout", bufs=4) as opool,
        ):
            wo_sb = wopool.tile([128, NHT * 512], BF16, tag="wo")
            for i in range(NHT):
                nc.sync.dma_start(out=wo_sb[:, i * 512:(i + 1) * 512],
                                  in_=wo[i * 128:(i + 1) * 128, :])
            agt = []
            for j in range(NREP):
                for r in range(N_CORES):
                    t = agpool.tile([128, S], BF16, tag=f"ag{j}_{r}", name=f"ag{j}_{r}")
                    nc.sync.dma_start(out=t[:],
                                      in_=ag_out[j][r * 128:(r + 1) * 128, :])
                    agt.append(t)
            for st in range(NST):
                po = psum.tile([128, 512], F32, tag="mm")
                for i in range(NHT):
                    nc.tensor.matmul(po[:], agt[i][:, st * 128:(st + 1) * 128],
                                     wo_sb[:, i * 512:(i + 1) * 512],
                                     start=(i == 0), stop=(i == NHT - 1))
                osb = opool.tile([128, 512], F32, tag="o")
                nc.scalar.copy(osb[:], po[:])
                nc.sync.dma_start(out=o_out[st * 128:(st + 1) * 128, :], in_=osb[:])


def prep_inputs(hidden_states, wq, wk, wv, wo, cos, sin, causal_mask=None):
    bf16 = ml_dtypes.bfloat16
    x = np.asarray(hidden_states, np.float32)[0]          # (S, HID)
    xT = np.ascontiguousarray(x.T).astype(bf16)           # (HID, S)
    wq_s = (np.asarray(wq, np.float32) / np.sqrt(HD)).astype(np.float32)
    cos2 = np.asarray(cos, np.float32)[0, 0]              # (S, 64)
    sin2 = np.asarray(sin, np.float32)[0, 0]
    cosT = np.ascontiguousarray(np.concatenate([cos2.T, cos2.T], 0))  # (128, S)
    sinT = np.ascontiguousarray(np.concatenate([sin2.T, sin2.T], 0))
    kl = np.arange(128)[:, None]
    ql = np.arange(128)[None, :]
    triT = (kl <= ql).astype(bf16)                        # allow k <= q
    ones_c = np.ones((128, 1), bf16)
    ones_r = np.ones((1, 128), np.float32)

    # wo reordered to match AllGather row order: row p = j*1024 + r*128 + d
    # corresponds to head (j*8+r), dim d  ->  wo column (j*8+r)*128 + d.
    j_ = np.arange(NREP)[:, None, None]
    r_ = np.arange(N_CORES)[None, :, None]
    d_ = np.arange(HD)[None, None, :]
    col_order = ((j_ * N_CORES + r_) * HD + d_).reshape(-1)
    woT_full = np.ascontiguousarray(
        np.asarray(wo, np.float32)[:, col_order].T).astype(bf16)  # (4096c, 4096hid)

    in_maps = []
    for c in range(N_CORES):
        heads = [jj * N_CORES + c for jj in range(NREP)]
        wq_rows = np.concatenate([wq_s[h * HD:(h + 1) * HD, :] for h in heads], 0)
        wqT_c = np.ascontiguousarray(wq_rows.T).astype(bf16)        # (HID, 512)
        wkT_c = np.ascontiguousarray(
            np.asarray(wk, np.float32)[c * HD:(c + 1) * HD, :].T).astype(bf16)
        wvT_c = np.ascontiguousarray(
            np.asarray(wv, np.float32)[c * HD:(c + 1) * HD, :].T).astype(bf16)
        woT_c = np.ascontiguousarray(woT_full[:, c * 512:(c + 1) * 512])
        in_maps.append(dict(xT=xT, wqT=wqT_c, wkT=wkT_c, wvT=wvT_c, woT=woT_c,
                            cosT=cosT, sinT=sinT, triT=triT,
                            ones_c=ones_c, ones_r=ones_r))
    return in_maps


def postprocess(results):
    out = np.empty((S, HID), np.float32)
    for c in range(N_CORES):
        out[:, c * 512:(c + 1) * 512] = results[c]["o_out"]
    return out[None]


def get_nc():
    if "nc" not in _CACHE:
        _CACHE["nc"] = build_nc()
    return _CACHE["nc"]


def kernel(hidden_states, wq, wk, wv, wo, cos, sin, causal_mask=None):
    nc = get_nc()
    in_maps = prep_inputs(hidden_states, wq, wk, wv, wo, cos, sin, causal_mask)
    res = run_bass_kernel_spmd(nc, in_maps, core_ids=list(range(N_CORES)))
    return postprocess(res.results)



# revision 5
# speedup vs baseline: 1.1932x; 1.1932x over previous
"""Tensor-parallel LlamaAttention (S=2048, HID=4096, NH=32, NKV=8) on 8 trn2 cores.

Sharding: core c owns q heads {c, c+8, c+16, c+24} (all share kv head c) and
kv head c.  Projections + attention fully local; per q-chunk C the normalized
avT slices of all 4 local heads are AllGathered (two half-gathers, j01/j23),
then each core computes its 512 o_proj output columns for that chunk's 4 seq
tiles (column-parallel wo) — so o_proj overlaps later attention chunks and
only the last chunk's gather is exposed.

Host packs all inputs partition-major so DMA lines are 8-32KB.
"""

import numpy as np
import ml_dtypes

import concourse.bacc as bacc
import concourse.tile as tile
import concourse.mybir as mybir
from concourse.bass_utils import run_bass_kernel_spmd

S = 2048
HID = 4096
NH = 32
NKV = 8
HD = 128
HALF = 64
N_CORES = 8
NREP = NH // NKV  # 4 q heads per core
NHT = HID // 128  # 32 hidden tiles
NST = S // 128    # 16 seq tiles
NSC = S // 512    # 4 seq chunks
BF16 = mybir.dt.bfloat16
F32 = mybir.dt.float32
F32R = mybir.dt.float32r

_CACHE = {}


def build_nc():
    nc = bacc.Bacc("TRN2", target_bir_lowering=False, debug=False,
                   num_devices=N_CORES)

    xP = nc.dram_tensor("xP", [128, NSC * NHT * 512], BF16,
                        kind="ExternalInput").ap()
    wqP = nc.dram_tensor("wqP", [128, NHT * 512], BF16,
                         kind="ExternalInput").ap()
    wkP = nc.dram_tensor("wkP", [128, NHT * 128], BF16,
                         kind="ExternalInput").ap()
    wvP = nc.dram_tensor("wvP", [128, NHT * 128], BF16,
                         kind="ExternalInput").ap()
    woP = nc.dram_tensor("woP", [128, NHT * 512], BF16,
                         kind="ExternalInput").ap()
    cosP = nc.dram_tensor("cosP", [HD, S], BF16, kind="ExternalInput").ap()
    sinmP = nc.dram_tensor("sinmP", [HD, S], BF16, kind="ExternalInput").ap()
    tri = nc.dram_tensor("triT", [128, 128], BF16, kind="ExternalInput").ap()
    eye = nc.dram_tensor("eyeT", [128, 128], BF16, kind="ExternalInput").ap()
    ones_c = nc.dram_tensor("ones_c", [128, 1], BF16, kind="ExternalInput").ap()
    ones_r = nc.dram_tensor("ones_r", [1, 128], F32, kind="ExternalInput").ap()

    o_out = nc.dram_tensor("o_out", [S, 512], F32, kind="ExternalOutput").ap()

    ag_in_a = [nc.dram_tensor(f"ag_in_a{c}", [128, 1024], BF16).ap()
               for c in range(NSC)]
    ag_in_b = [nc.dram_tensor(f"ag_in_b{c}", [128, 1024], BF16).ap()
               for c in range(NSC)]
    ag_out_a = [nc.dram_tensor(f"ag_out_a{c}", [N_CORES * 128, 1024], BF16,
                               addr_space="Shared").ap() for c in range(NSC)]
    ag_out_b = [nc.dram_tensor(f"ag_out_b{c}", [N_CORES * 128, 1024], BF16,
                               addr_space="Shared").ap() for c in range(NSC)]

    with tile.TileContext(nc) as tc:
        _body(nc, tc, xP, wqP, wkP, wvP, woP, cosP, sinmP, tri, eye,
              ones_c, ones_r, o_out, ag_in_a, ag_in_b, ag_out_a, ag_out_b)
    nc.compile()
    return nc


def _body(nc, tc, xP, wqP, wkP, wvP, woP, cosP, sinmP, tri, eye,
          ones_c, ones_r, o_out, ag_in_a, ag_in_b, ag_out_a, ag_out_b):
    with tc.tile_pool(name="consts", bufs=1) as cpool:
        tri_sb = cpool.tile([128, 128], BF16, tag="tri")
        eye_sb = cpool.tile([128, 128], BF16, tag="eye")
        onc_sb = cpool.tile([128, 1], BF16, tag="onc")
        onr_sb = cpool.tile([1, 128], F32, tag="onr")
        nc.sync.dma_start(out=tri_sb[:], in_=tri[:])
        nc.sync.dma_start(out=eye_sb[:], in_=eye[:])
        nc.sync.dma_start(out=onc_sb[:], in_=ones_c[:])
        nc.sync.dma_start(out=onr_sb[:], in_=ones_r[:])

        with tc.tile_pool(name="qkv", bufs=1) as qkvpool:
            qT_sb = [qkvpool.tile([HD, S], BF16, tag=f"qT{j}", name=f"qT{j}")
                     for j in range(NREP)]
            kT_sb = qkvpool.tile([HD, S], BF16, tag="kT")
            v_sb = qkvpool.tile([128, S], BF16, tag="v")  # [s-in-tile, d]

            with (
                tc.tile_pool(name="rconsts", bufs=1) as rcpool,
                tc.tile_pool(name="wproj", bufs=1) as wpool,
                tc.tile_pool(name="xc", bufs=2) as xpool,
                tc.tile_pool(name="rope", bufs=2) as rpool,
                tc.tile_pool(name="pmm1", bufs=6, space="PSUM") as pmm1,
                tc.tile_pool(name="ptr", bufs=2, space="PSUM") as ptrp,
            ):
                _phase1(nc, tc, xP, wqP, wkP, wvP, cosP, sinmP, eye_sb,
                        qT_sb, kT_sb, v_sb, rcpool, wpool, xpool, rpool,
                        pmm1, ptrp)

            with (
                tc.tile_pool(name="wo", bufs=1) as wopool,
                tc.tile_pool(name="av", bufs=2) as avpool,
                tc.tile_pool(name="probs", bufs=16) as ppool,
                tc.tile_pool(name="srec", bufs=2) as srpool,
                tc.tile_pool(name="sbrd", bufs=2) as sbpool,
                tc.tile_pool(name="ag", bufs=24) as agpool,
                tc.tile_pool(name="oout", bufs=4) as opool,
                tc.tile_pool(name="pmm2", bufs=4, space="PSUM") as pmm2,
                tc.tile_pool(name="psav", bufs=2, space="PSUM") as psav,
                tc.tile_pool(name="psrs", bufs=2, space="PSUM") as psrs,
            ):
                _phase23(nc, tc, woP, o_out, ag_in_a, ag_in_b,
                         ag_out_a, ag_out_b, tri_sb, onc_sb, onr_sb,
                         qT_sb, kT_sb, v_sb, wopool, avpool, ppool,
                         srpool, sbpool, agpool, opool, pmm2, psav, psrs)


def _phase1(nc, tc, xP, wqP, wkP, wvP, cosP, sinmP, eye_sb,
            qT_sb, kT_sb, v_sb, rcpool, wpool, xpool, rpool, pmm1, ptrp):
    cos_sb = rcpool.tile([HD, S], BF16, tag="cos")
    sinm_sb = rcpool.tile([HD, S], BF16, tag="sinm")
    nc.sync.dma_start(out=cos_sb[:], in_=cosP[:])
    nc.scalar.dma_start(out=sinm_sb[:], in_=sinmP[:])

    wq_sb = wpool.tile([128, NHT * 512], BF16, tag="wq")
    wk_sb = wpool.tile([128, NHT * 128], BF16, tag="wk")
    wv_sb = wpool.tile([128, NHT * 128], BF16, tag="wv")
    H = NHT * 256
    nc.sync.dma_start(out=wq_sb[:, 0:H], in_=wqP[:, 0:H])
    nc.scalar.dma_start(out=wq_sb[:, H:2 * H], in_=wqP[:, H:2 * H])
    nc.sync.dma_start(out=wk_sb[:], in_=wkP[:])
    nc.scalar.dma_start(out=wv_sb[:], in_=wvP[:])

    def _rope(dst, pp, sc):
        # dst[:, sc] = rot(pp): [0:64] = x1*c - x2*s ; [64:128] = x2*c + x1*s
        # cos_sb = [c; c], sinm_sb = [-s; s] (signs baked host-side)
        pq_sb = rpool.tile([128, 512], BF16, tag="pq")
        nc.scalar.copy(pq_sb[:], pp[:])
        sw = rpool.tile([128, 512], BF16, tag="sw")
        nc.vector.tensor_scalar_mul(sw[0:HALF, :], pq_sb[HALF:128, :], 1.0)
        nc.vector.tensor_scalar_mul(sw[HALF:128, :], pq_sb[0:HALF, :], 1.0)
        a = rpool.tile([128, 512], BF16, tag="ra")
        b = rpool.tile([128, 512], BF16, tag="rb")
        nc.vector.tensor_mul(a[:], pq_sb[:], cos_sb[:, sc])
        nc.vector.tensor_mul(b[:], sw[:], sinm_sb[:, sc])
        nc.vector.tensor_add(dst[:, sc], a[:], b[:])

    for cs in range(NSC):
        sc = slice(cs * 512, (cs + 1) * 512)
        x_sb = xpool.tile([128, NHT * 512], BF16, tag="x", name=f"x{cs}")
        Q = NHT * 128  # 4096 cols per quarter
        for part in range(4):
            eng = nc.sync if part % 2 == 0 else nc.scalar
            eng.dma_start(out=x_sb[:, part * Q:(part + 1) * Q],
                          in_=xP[:, cs * NHT * 512 + part * Q:
                                 cs * NHT * 512 + (part + 1) * Q])

        for j in range(NREP):
            pq = pmm1.tile([128, 512], F32, tag="mm", name=f"pq{cs}_{j}")
            for h in range(NHT):
                nc.tensor.matmul(
                    pq[:],
                    wq_sb[:, h * 512 + j * 128: h * 512 + (j + 1) * 128],
                    x_sb[:, h * 512:(h + 1) * 512],
                    start=(h == 0), stop=(h == NHT - 1))
            _rope(qT_sb[j], pq, sc)

        pk = pmm1.tile([128, 512], F32, tag="mm", name=f"pk{cs}")
        for h in range(NHT):
            nc.tensor.matmul(pk[:], wk_sb[:, h * 128:(h + 1) * 128],
                             x_sb[:, h * 512:(h + 1) * 512],
                             start=(h == 0), stop=(h == NHT - 1))
        _rope(kT_sb, pk, sc)

        # vT chunk [d, s'] with N=512 matmuls, then PE-transpose to [s, d]
        pv = pmm1.tile([128, 512], F32, tag="mm", name=f"pv{cs}")
        for h in range(NHT):
            nc.tensor.matmul(pv[:], wv_sb[:, h * 128:(h + 1) * 128],
                             x_sb[:, h * 512:(h + 1) * 512],
                             start=(h == 0), stop=(h == NHT - 1))
        vt_sb = rpool.tile([128, 512], BF16, tag="vt", name=f"vt{cs}")
        nc.scalar.copy(vt_sb[:], pv[:])
        for tl in range(4):
            pt_ps = ptrp.tile([128, 128], BF16, tag="tr", name=f"vtr{cs}_{tl}")
            nc.tensor.transpose(pt_ps[:], vt_sb[:, tl * 128:(tl + 1) * 128],
                                eye_sb[:])
            t0 = (cs * 4 + tl) * 128
            nc.scalar.copy(v_sb[:, t0:t0 + 128], pt_ps[:])


def _phase23(nc, tc, woP, o_out, ag_in_a, ag_in_b, ag_out_a, ag_out_b,
             tri_sb, onc_sb, onr_sb, qT_sb, kT_sb, v_sb, wopool, avpool,
             ppool, srpool, sbpool, agpool, opool, pmm2, psav, psrs):
    Exp = mybir.ActivationFunctionType.Exp
    DEPTH = 3
    RG = [list(range(N_CORES))]

    wo_sb = wopool.tile([128, NHT * 512], BF16, tag="wo")
    H = NHT * 256
    nc.sync.dma_start(out=wo_sb[:, 0:H], in_=woP[:, 0:H])
    nc.scalar.dma_start(out=wo_sb[:, H:2 * H], in_=woP[:, H:2 * H])

    av_tiles = [avpool.tile([128, 2048], BF16, tag="av", name=f"av{C}")
                for C in range(NSC)]

    state = {"recip": None, "norm": None}
    rrecs = {}

    def make_recip(j, C, prs):
        def f():
            rrec = srpool.tile([1, 512], F32, tag="rrec", name=f"rrec{j}_{C}")
            nc.vector.reciprocal_approx_fast(rrec[:], prs[:])
            rrecs[(j, C)] = rrec
        return f

    def make_norm(j, C, pav):
        def f():
            rrec = rrecs.pop((j, C))
            pb = pmm2.tile([128, 512], F32, tag="mm", name=f"pb{j}_{C}")
            nc.tensor.matmul(pb[:], onr_sb[:], rrec[:], start=True, stop=True)
            bsb = sbpool.tile([128, 512], F32, tag="bsb", name=f"bsb{j}_{C}")
            nc.vector.tensor_scalar_mul(bsb[:], pb[:], 1.0)
            nc.vector.tensor_mul(av_tiles[C][:, j * 512:(j + 1) * 512],
                                 pav[:], bsb[:])
            if j == 1:
                nc.sync.dma_start(out=ag_in_a[C][:],
                                  in_=av_tiles[C][:, 0:1024])
                nc.gpsimd.collective_compute(
                    "AllGather", mybir.AluOpType.bypass, replica_groups=RG,
                    ins=[ag_in_a[C][:]], outs=[ag_out_a[C][:]])
            if j == 3:
                nc.sync.dma_start(out=ag_in_b[C][:],
                                  in_=av_tiles[C][:, 1024:2048])
                nc.gpsimd.collective_compute(
                    "AllGather", mybir.AluOpType.bypass, replica_groups=RG,
                    ins=[ag_in_b[C][:]], outs=[ag_out_b[C][:]])
        return f

    def run_carry(kind):
        if state[kind] is not None:
            state[kind]()
            state[kind] = None

    def attn_unit(j, C):
        qc0 = C * 512
        nkt = 4 * C + 4
        pav = psav.tile([128, 512], F32, tag="av", name=f"pav{j}_{C}")
        prs = psrs.tile([1, 512], F32, tag="rs", name=f"prs{j}_{C}")
        pend = []

        def drain_one():
            kt2, pt2 = pend.pop(0)
            nc.tensor.matmul(prs[:], onc_sb[:], pt2[:],
                             start=(kt2 == 0), stop=(kt2 == nkt - 1))
            nc.tensor.matmul(pav[:], v_sb[:, kt2 * 128:(kt2 + 1) * 128],
                             pt2[:], start=(kt2 == 0), stop=(kt2 == nkt - 1))

        for kt in range(nkt):
            off = max(0, (kt - 4 * C) * 128)
            ps = pmm2.tile([128, 512], F32, tag="mm", name=f"ps{j}_{C}_{kt}")
            nc.tensor.matmul(ps[:, off:512],
                             kT_sb[:, kt * 128:(kt + 1) * 128],
                             qT_sb[j][:, qc0 + off: qc0 + 512],
                             start=True, stop=True)
            pt = ppool.tile([128, 512], BF16, tag="pt", name=f"pt{j}_{C}_{kt}")
            if off > 0:
                nc.vector.memset(pt[:, 0:off], 0.0)
            nc.scalar.activation(pt[:, off:512], ps[:, off:512], Exp)
            if kt >= 4 * C:
                nc.vector.tensor_mul(pt[:, off:off + 128],
                                     pt[:, off:off + 128], tri_sb[:])
            pend.append((kt, pt))
            if kt == 1:
                run_carry("recip")
            if kt == 3:
                run_carry("norm")
            if len(pend) > DEPTH:
                drain_one()
        while pend:
            drain_one()
        state["recip"] = make_recip(j, C, prs)
        state["norm"] = make_norm(j, C, pav)

    def ph3(C):
        ta, tb = [], []
        for r in range(N_CORES):
            t = agpool.tile([128, 1024], BF16, tag="ag", name=f"aga{C}_{r}")
            eng = nc.sync if r % 2 == 0 else nc.scalar
            eng.dma_start(out=t[:], in_=ag_out_a[C][r * 128:(r + 1) * 128, :])
            ta.append(t)
        for r in range(N_CORES):
            t = agpool.tile([128, 1024], BF16, tag="ag", name=f"agb{C}_{r}")
            eng = nc.sync if r % 2 == 0 else nc.scalar
            eng.dma_start(out=t[:], in_=ag_out_b[C][r * 128:(r + 1) * 128, :])
            tb.append(t)
        for stl in range(4):
            st = 4 * C + stl
            po = pmm2.tile([128, 512], F32, tag="mm", name=f"po{st}")
            n = 0
            for jgrp in range(NREP):
                src = ta if jgrp < 2 else tb
                jj = jgrp % 2
                for r in range(N_CORES):
                    nc.tensor.matmul(
                        po[:],
                        src[r][:, jj * 512 + stl * 128:
                               jj * 512 + (stl + 1) * 128],
                        wo_sb[:, (jgrp * 8 + r) * 512:(jgrp * 8 + r + 1) * 512],
                        start=(n == 0), stop=(n == NREP * N_CORES - 1))
                    n += 1
            osb = opool.tile([128, 512], F32, tag="o", name=f"o{st}")
            nc.vector.tensor_scalar_mul(osb[:], po[:], 1.0)
            nc.sync.dma_start(out=o_out[st * 128:(st + 1) * 128, :],
                              in_=osb[:])

    for C in range(NSC):
        for j in range(NREP):
            attn_unit(j, C)
            if C >= 1 and j == 2:
                ph3(C - 1)
    run_carry("recip")
    run_carry("norm")
    ph3(NSC - 1)


def prep_inputs(hidden_states, wq, wk, wv, wo, cos, sin, causal_mask=None):
    bf16 = ml_dtypes.bfloat16
    x = np.asarray(hidden_states, np.float32)[0]          # (S, HID)
    xT = np.ascontiguousarray(x.T)                        # (HID, S)
    # chunk-major pack: xP[p, cs*16384 + h*512 + s'] = xT[h*128+p, cs*512+s']
    xP = np.ascontiguousarray(
        xT.reshape(NHT, 128, NSC, 512).transpose(1, 2, 0, 3)
        .reshape(128, NSC * NHT * 512)).astype(bf16)
    wq_s = np.asarray(wq, np.float32) / np.sqrt(HD)
    cos2 = np.asarray(cos, np.float32)[0, 0]              # (S, 64)
    sin2 = np.asarray(sin, np.float32)[0, 0]
    cosP = np.ascontiguousarray(
        np.concatenate([cos2.T, cos2.T], 0)).astype(bf16)  # [c; c]
    sinmP = np.ascontiguousarray(
        np.concatenate([-sin2.T, sin2.T], 0)).astype(bf16)  # [-s; s]
    kl = np.arange(128)[:, None]
    ql = np.arange(128)[None, :]
    triT = (kl <= ql).astype(bf16)
    eyeT = np.eye(128, dtype=np.float32).astype(bf16)
    ones_c = np.ones((128, 1), bf16)
    ones_r = np.ones((1, 128), np.float32)

    # wo reordered so row p = (j*8+r)*128 + d maps head (j*8+r), dim d
    j_ = np.arange(NREP)[:, None, None]
    r_ = np.arange(N_CORES)[None, :, None]
    d_ = np.arange(HD)[None, None, :]
    col_order = ((j_ * N_CORES + r_) * HD + d_).reshape(-1)
    woT_full = np.ascontiguousarray(
        np.asarray(wo, np.float32)[:, col_order].T)       # (4096 avrow, 4096)

    def pack_w(wT, ncols):  # (HID, ncols) -> (128, NHT*ncols), col h*ncols+c
        return np.ascontiguousarray(
            wT.reshape(NHT, 128, ncols).transpose(1, 0, 2)
            .reshape(128, NHT * ncols)).astype(bf16)

    in_maps = []
    for c in range(N_CORES):
        heads = [jj * N_CORES + c for jj in range(NREP)]
        wq_rows = np.concatenate([wq_s[h * HD:(h + 1) * HD, :] for h in heads],
                                 0)                        # (512, HID)
        wqP = pack_w(np.ascontiguousarray(wq_rows.T), 512)
        wkT_c = np.ascontiguousarray(
            np.asarray(wk, np.float32)[c * HD:(c + 1) * HD, :].T)
        wvT_c = np.ascontiguousarray(
            np.asarray(wv, np.float32)[c * HD:(c + 1) * HD, :].T)
        wkP = pack_w(wkT_c, 128)
        wvP = pack_w(wvT_c, 128)
        woP = pack_w(np.ascontiguousarray(
            woT_full[:, c * 512:(c + 1) * 512]), 512)
        in_maps.append(dict(xP=xP, wqP=wqP, wkP=wkP, wvP=wvP, woP=woP,
                            cosP=cosP, sinmP=sinmP, triT=triT, eyeT=eyeT,
                            ones_c=ones_c, ones_r=ones_r))
    return in_maps


def postprocess(results):
    out = np.empty((S, HID), np.float32)
    for c in range(N_CORES):
        out[:, c * 512:(c + 1) * 512] = results[c]["o_out"]
    return out[None]


def get_nc():
    if "nc" not in _CACHE:
        _CACHE["nc"] = build_nc()
    return _CACHE["nc"]


def kernel(hidden_states, wq, wk, wv, wo, cos, sin, causal_mask=None):
    nc = get_nc()
    in_maps = prep_inputs(hidden_states, wq, wk, wv, wo, cos, sin, causal_mask)
    res = run_bass_kernel_spmd(nc, in_maps, core_ids=list(range(N_CORES)))
    return postprocess(res.results)


# revision 13
# speedup vs baseline: 1.2177x; 1.0205x over previous
"""Tensor-parallel LlamaAttention (S=2048, HID=4096, NH=32, NKV=8) on 8 trn2 cores.

Sharding: core c owns q heads {c, c+8, c+16, c+24} (all share kv head c) and
kv head c.  Projections + attention fully local; per q-chunk C the normalized
avT slices of all 4 local heads are AllGathered (two half-gathers, j01/j23),
then each core computes its 512 o_proj output columns for that chunk's 4 seq
tiles (column-parallel wo) — so o_proj overlaps later attention chunks and
only the last chunk's gather is exposed.

Host packs all inputs partition-major so DMA lines are 8-32KB.
"""

import numpy as np
import ml_dtypes

import concourse.bacc as bacc
import concourse.tile as tile
import concourse.mybir as mybir
from concourse.bass_utils import run_bass_kernel_spmd

S = 2048
HID = 4096
NH = 32
NKV = 8
HD = 128
HALF = 64
N_CORES = 8
NREP = NH // NKV  # 4 q heads per core
NHT = HID // 128  # 32 hidden tiles
NST = S // 128    # 16 seq tiles
NSC = S // 512    # 4 seq chunks
BF16 = mybir.dt.bfloat16
F32 = mybir.dt.float32
F32R = mybir.dt.float32r

_CACHE = {}


def build_nc():
    nc = bacc.Bacc("TRN2", target_bir_lowering=False, debug=False,
                   num_devices=N_CORES)

    xP = nc.dram_tensor("xP", [128, NSC * NHT * 512], BF16,
                        kind="ExternalInput").ap()
    wqP = nc.dram_tensor("wqP", [128, NHT * 512], BF16,
                         kind="ExternalInput").ap()
    wkP = nc.dram_tensor("wkP", [128, NHT * 128], BF16,
                         kind="ExternalInput").ap()
    wvP = nc.dram_tensor("wvP", [128, NHT * 128], BF16,
                         kind="ExternalInput").ap()
    woP = nc.dram_tensor("woP", [128, NHT * 512], BF16,
                         kind="ExternalInput").ap()
    cosP = nc.dram_tensor("cosP", [HD, S], BF16, kind="ExternalInput").ap()
    sinmP = nc.dram_tensor("sinmP", [HD, S], BF16, kind="ExternalInput").ap()
    tri = nc.dram_tensor("triT", [128, 128], BF16, kind="ExternalInput").ap()
    eye = nc.dram_tensor("eyeT", [128, 128], BF16, kind="ExternalInput").ap()
    ones_c = nc.dram_tensor("ones_c", [128, 1], BF16, kind="ExternalInput").ap()
    ones_r = nc.dram_tensor("ones_r", [1, 128], F32, kind="ExternalInput").ap()

    o_out = nc.dram_tensor("o_out", [S, 512], F32, kind="ExternalOutput").ap()

    ag_in_a = [nc.dram_tensor(f"ag_in_a{c}", [128, 1024], BF16).ap()
               for c in range(NSC)]
    ag_in_b = [nc.dram_tensor(f"ag_in_b{c}", [128, 1024], BF16).ap()
               for c in range(NSC)]
    ag_out_a = [nc.dram_tensor(f"ag_out_a{c}", [N_CORES * 128, 1024], BF16,
                               addr_space="Shared").ap() for c in range(NSC)]
    ag_out_b = [nc.dram_tensor(f"ag_out_b{c}", [N_CORES * 128, 1024], BF16,
                               addr_space="Shared").ap() for c in range(NSC)]

    with tile.TileContext(nc) as tc:
        _body(nc, tc, xP, wqP, wkP, wvP, woP, cosP, sinmP, tri, eye,
              ones_c, ones_r, o_out, ag_in_a, ag_in_b, ag_out_a, ag_out_b)
    nc.compile()
    return nc


def _body(nc, tc, xP, wqP, wkP, wvP, woP, cosP, sinmP, tri, eye,
          ones_c, ones_r, o_out, ag_in_a, ag_in_b, ag_out_a, ag_out_b):
    with tc.tile_pool(name="consts", bufs=1) as cpool:
        tri_sb = cpool.tile([128, 128], BF16, tag="tri")
        eye_sb = cpool.tile([128, 128], BF16, tag="eye")
        onc_sb = cpool.tile([128, 1], BF16, tag="onc")
        onr_sb = cpool.tile([1, 128], F32, tag="onr")
        nc.sync.dma_start(out=tri_sb[:], in_=tri[:])
        nc.sync.dma_start(out=eye_sb[:], in_=eye[:])
        nc.sync.dma_start(out=onc_sb[:], in_=ones_c[:])
        nc.sync.dma_start(out=onr_sb[:], in_=ones_r[:])

        with tc.tile_pool(name="qkv", bufs=1) as qkvpool:
            qT_sb = [qkvpool.tile([HD, S], BF16, tag=f"qT{j}", name=f"qT{j}")
                     for j in range(NREP)]
            kT_sb = qkvpool.tile([HD, S], BF16, tag="kT")
            v_sb = qkvpool.tile([128, S], BF16, tag="v")  # [s-in-tile, d]

            with (
                tc.tile_pool(name="rconsts", bufs=1) as rcpool,
                tc.tile_pool(name="wproj", bufs=1) as wpool,
                tc.tile_pool(name="xc", bufs=2) as xpool,
                tc.tile_pool(name="rope", bufs=2) as rpool,
                tc.tile_pool(name="pmm1", bufs=6, space="PSUM") as pmm1,
                tc.tile_pool(name="ptr", bufs=2, space="PSUM") as ptrp,
            ):
                _phase1(nc, tc, xP, wqP, wkP, wvP, cosP, sinmP, eye_sb,
                        qT_sb, kT_sb, v_sb, rcpool, wpool, xpool, rpool,
                        pmm1, ptrp)

            with (
                tc.tile_pool(name="wo", bufs=1) as wopool,
                tc.tile_pool(name="av", bufs=2) as avpool,
                tc.tile_pool(name="probs", bufs=16) as ppool,
                tc.tile_pool(name="srec", bufs=2) as srpool,
                tc.tile_pool(name="sbrd", bufs=2) as sbpool,
                tc.tile_pool(name="ag", bufs=16) as agpool,
                tc.tile_pool(name="oout", bufs=4) as opool,
                tc.tile_pool(name="pmm2", bufs=4, space="PSUM") as pmm2,
                tc.tile_pool(name="psav", bufs=2, space="PSUM") as psav,
                tc.tile_pool(name="psrs", bufs=2, space="PSUM") as psrs,
            ):
                _phase23(nc, tc, woP, o_out, ag_in_a, ag_in_b,
                         ag_out_a, ag_out_b, tri_sb, onc_sb, onr_sb,
                         qT_sb, kT_sb, v_sb, wopool, avpool, ppool,
                         srpool, sbpool, agpool, opool, pmm2, psav, psrs)


def _phase1(nc, tc, xP, wqP, wkP, wvP, cosP, sinmP, eye_sb,
            qT_sb, kT_sb, v_sb, rcpool, wpool, xpool, rpool, pmm1, ptrp):
    def dma_split(dst, src, n):
        # n parallel slices across the two HWDGE queues' rings
        w = dst.shape[-1] // n
        for i in range(n):
            eng = nc.sync if i % 2 == 0 else nc.scalar
            eng.dma_start(out=dst[:, i * w:(i + 1) * w],
                          in_=src[:, i * w:(i + 1) * w])

    cos_sb = rcpool.tile([HD, S], BF16, tag="cos")
    sinm_sb = rcpool.tile([HD, S], BF16, tag="sinm")
    dma_split(cos_sb, cosP, 2)
    dma_split(sinm_sb, sinmP, 2)

    wq_sb = wpool.tile([128, NHT * 512], BF16, tag="wq")
    wk_sb = wpool.tile([128, NHT * 128], BF16, tag="wk")
    wv_sb = wpool.tile([128, NHT * 128], BF16, tag="wv")
    dma_split(wq_sb, wqP, 8)
    dma_split(wk_sb, wkP, 2)
    dma_split(wv_sb, wvP, 2)

    def _rope(dst, pp, sc):
        # dst[:, sc] = rot(pp): [0:64] = x1*c - x2*s ; [64:128] = x2*c + x1*s
        # cos_sb = [c; c], sinm_sb = [-s; s] (signs baked host-side)
        pq_sb = rpool.tile([128, 512], BF16, tag="pq")
        nc.scalar.copy(pq_sb[:], pp[:])
        sw = rpool.tile([128, 512], BF16, tag="sw")
        nc.vector.tensor_scalar_mul(sw[0:HALF, :], pq_sb[HALF:128, :], 1.0)
        nc.vector.tensor_scalar_mul(sw[HALF:128, :], pq_sb[0:HALF, :], 1.0)
        a = rpool.tile([128, 512], BF16, tag="ra")
        b = rpool.tile([128, 512], BF16, tag="rb")
        nc.vector.tensor_mul(a[:], pq_sb[:], cos_sb[:, sc])
        nc.vector.tensor_mul(b[:], sw[:], sinm_sb[:, sc])
        nc.vector.tensor_add(dst[:, sc], a[:], b[:])

    for cs in range(NSC):
        sc = slice(cs * 512, (cs + 1) * 512)
        x_sb = xpool.tile([128, NHT * 512], BF16, tag="x", name=f"x{cs}")
        Q = NHT * 32  # 1024 cols per slice, 16 slices
        for part in range(16):
            eng = nc.sync if part % 2 == 0 else nc.scalar
            eng.dma_start(out=x_sb[:, part * Q:(part + 1) * Q],
                          in_=xP[:, cs * NHT * 512 + part * Q:
                                 cs * NHT * 512 + (part + 1) * Q])

        for j in range(NREP):
            pq = pmm1.tile([128, 512], F32, tag="mm", name=f"pq{cs}_{j}")
            for h in range(NHT):
                nc.tensor.matmul(
                    pq[:],
                    wq_sb[:, h * 512 + j * 128: h * 512 + (j + 1) * 128],
                    x_sb[:, h * 512:(h + 1) * 512],
                    start=(h == 0), stop=(h == NHT - 1))
            _rope(qT_sb[j], pq, sc)

        pk = pmm1.tile([128, 512], F32, tag="mm", name=f"pk{cs}")
        for h in range(NHT):
            nc.tensor.matmul(pk[:], wk_sb[:, h * 128:(h + 1) * 128],
                             x_sb[:, h * 512:(h + 1) * 512],
                             start=(h == 0), stop=(h == NHT - 1))
        _rope(kT_sb, pk, sc)

        # vT chunk [d, s'] with N=512 matmuls, then PE-transpose to [s, d]
        pv = pmm1.tile([128, 512], F32, tag="mm", name=f"pv{cs}")
        for h in range(NHT):
            nc.tensor.matmul(pv[:], wv_sb[:, h * 128:(h + 1) * 128],
                             x_sb[:, h * 512:(h + 1) * 512],
                             start=(h == 0), stop=(h == NHT - 1))
        vt_sb = rpool.tile([128, 512], BF16, tag="vt", name=f"vt{cs}")
        nc.scalar.copy(vt_sb[:], pv[:])
        for tl in range(4):
            pt_ps = ptrp.tile([128, 128], BF16, tag="tr", name=f"vtr{cs}_{tl}")
            nc.tensor.transpose(pt_ps[:], vt_sb[:, tl * 128:(tl + 1) * 128],
                                eye_sb[:])
            t0 = (cs * 4 + tl) * 128
            nc.scalar.copy(v_sb[:, t0:t0 + 128], pt_ps[:])


def _phase23(nc, tc, woP, o_out, ag_in_a, ag_in_b, ag_out_a, ag_out_b,
             tri_sb, onc_sb, onr_sb, qT_sb, kT_sb, v_sb, wopool, avpool,
             ppool, srpool, sbpool, agpool, opool, pmm2, psav, psrs):
    Exp = mybir.ActivationFunctionType.Exp
    DEPTH = 3
    RG = [list(range(N_CORES))]

    wo_sb = wopool.tile([128, NHT * 512], BF16, tag="wo")
    W8 = NHT * 64
    for i in range(8):
        eng = nc.sync if i % 2 == 0 else nc.scalar
        eng.dma_start(out=wo_sb[:, i * W8:(i + 1) * W8],
                      in_=woP[:, i * W8:(i + 1) * W8])

    av_tiles = [avpool.tile([128, 2048], BF16, tag="av", name=f"av{C}")
                for C in range(NSC)]

    state = {"recip": None, "norm": None}
    rrecs = {}
    agt = {}  # (C, 'a'|'b') -> list of 8 SBUF tiles, loaded as AGs land

    def load_gather(C, half, ag_out):
        ts = []
        for r in range(N_CORES):
            t = agpool.tile([128, 1024], BF16, tag="ag",
                            name=f"ag{half}{C}_{r}")
            eng = nc.sync if r % 2 == 0 else nc.scalar
            eng.dma_start(out=t[:], in_=ag_out[r * 128:(r + 1) * 128, :])
            ts.append(t)
        agt[(C, half)] = ts

    def make_recip(j, C, prs):
        def f():
            rrec = srpool.tile([1, 512], F32, tag="rrec", name=f"rrec{j}_{C}")
            nc.vector.reciprocal_approx_fast(rrec[:], prs[:])
            rrecs[(j, C)] = rrec
        return f

    def make_norm(j, C, pav):
        def f():
            rrec = rrecs.pop((j, C))
            pb = pmm2.tile([128, 512], F32, tag="mm", name=f"pb{j}_{C}")
            nc.tensor.matmul(pb[:], onr_sb[:], rrec[:], start=True, stop=True)
            bsb = sbpool.tile([128, 512], F32, tag="bsb", name=f"bsb{j}_{C}")
            nc.vector.tensor_scalar_mul(bsb[:], pb[:], 1.0)
            nc.vector.tensor_mul(av_tiles[C][:, j * 512:(j + 1) * 512],
                                 pav[:], bsb[:])
            if j == 1:
                nc.sync.dma_start(out=ag_in_a[C][:],
                                  in_=av_tiles[C][:, 0:1024])
                nc.gpsimd.collective_compute(
                    "AllGather", mybir.AluOpType.bypass, replica_groups=RG,
                    ins=[ag_in_a[C][:]], outs=[ag_out_a[C][:]])
            if j == 3:
                nc.sync.dma_start(out=ag_in_b[C][:],
                                  in_=av_tiles[C][:, 1024:2048])
                nc.gpsimd.collective_compute(
                    "AllGather", mybir.AluOpType.bypass, replica_groups=RG,
                    ins=[ag_in_b[C][:]], outs=[ag_out_b[C][:]])
        return f

    def run_carry(kind):
        if state[kind] is not None:
            state[kind]()
            state[kind] = None

    def attn_unit(j, C):
        qc0 = C * 512
        nkt = 4 * C + 4
        pav = psav.tile([128, 512], F32, tag="av", name=f"pav{j}_{C}")
        prs = psrs.tile([1, 512], F32, tag="rs", name=f"prs{j}_{C}")
        pend = []

        def drain_one():
            kt2, pt2 = pend.pop(0)
            nc.tensor.matmul(prs[:], onc_sb[:], pt2[:],
                             start=(kt2 == 0), stop=(kt2 == nkt - 1))
            nc.tensor.matmul(pav[:], v_sb[:, kt2 * 128:(kt2 + 1) * 128],
                             pt2[:], start=(kt2 == 0), stop=(kt2 == nkt - 1))

        for kt in range(nkt):
            off = max(0, (kt - 4 * C) * 128)
            ps = pmm2.tile([128, 512], F32, tag="mm", name=f"ps{j}_{C}_{kt}")
            nc.tensor.matmul(ps[:, off:512],
                             kT_sb[:, kt * 128:(kt + 1) * 128],
                             qT_sb[j][:, qc0 + off: qc0 + 512],
                             start=True, stop=True)
            pt = ppool.tile([128, 512], BF16, tag="pt", name=f"pt{j}_{C}_{kt}")
            if off > 0:
                nc.vector.memset(pt[:, 0:off], 0.0)
            nc.scalar.activation(pt[:, off:512], ps[:, off:512], Exp)
            if kt >= 4 * C:
                nc.vector.tensor_mul(pt[:, off:off + 128],
                                     pt[:, off:off + 128], tri_sb[:])
            pend.append((kt, pt))
            if kt == 1:
                run_carry("recip")
            if kt == 3:
                run_carry("norm")
            if len(pend) > DEPTH:
                drain_one()
        while pend:
            drain_one()
        state["recip"] = make_recip(j, C, prs)
        state["norm"] = make_norm(j, C, pav)

    def ph3_half(C, half, pos):
        # accumulate 16 matmuls per st tile; pos: list of 4 open po tiles
        load_gather(C, half, (ag_out_a if half == "a" else ag_out_b)[C])
        ts = agt.pop((C, half))
        base_j = 0 if half == "a" else 2
        for stl in range(4):
            st = 4 * C + stl
            if half == "a":
                pos.append(pmm2.tile([128, 512], F32, tag="mm",
                                     name=f"po{st}"))
            po = pos[stl]
            n = 0
            for jj in range(2):
                jgrp = base_j + jj
                for r in range(N_CORES):
                    nc.tensor.matmul(
                        po[:],
                        ts[r][:, jj * 512 + stl * 128:
                              jj * 512 + (stl + 1) * 128],
                        wo_sb[:, (jgrp * 8 + r) * 512:(jgrp * 8 + r + 1) * 512],
                        start=(half == "a" and n == 0),
                        stop=(half == "b" and n == 15))
                    n += 1
            if half == "b":
                osb = opool.tile([128, 512], F32, tag="o", name=f"o{st}")
                nc.vector.tensor_scalar_mul(osb[:], po[:], 1.0)
                nc.sync.dma_start(out=o_out[st * 128:(st + 1) * 128, :],
                                  in_=osb[:])

    def ph3(C):
        pos = []
        ph3_half(C, "a", pos)
        ph3_half(C, "b", pos)

    # ph3(C) placement: ≥ ~22us after AG_b(C) fires (which happens in
    # unit (C+1, 0)'s norm carry), so the gathers have landed.
    PH3_AT = {(2, 0): 0, (2, 3): 1, (3, 2): 2}
    for C in range(NSC):
        for j in range(NREP):
            attn_unit(j, C)
            if (C, j) in PH3_AT:
                ph3(PH3_AT[(C, j)])
    run_carry("recip")
    run_carry("norm")
    pos = []
    ph3_half(NSC - 1, "a", pos)
    ph3_half(NSC - 1, "b", pos)


def prep_inputs(hidden_states, wq, wk, wv, wo, cos, sin, causal_mask=None):
    bf16 = ml_dtypes.bfloat16
    x = np.asarray(hidden_states, np.float32)[0]          # (S, HID)
    xT = np.ascontiguousarray(x.T)                        # (HID, S)
    # chunk-major pack: xP[p, cs*16384 + h*512 + s'] = xT[h*128+p, cs*512+s']
    xP = np.ascontiguousarray(
        xT.reshape(NHT, 128, NSC, 512).transpose(1, 2, 0, 3)
        .reshape(128, NSC * NHT * 512)).astype(bf16)
    wq_s = np.asarray(wq, np.float32) / np.sqrt(HD)
    cos2 = np.asarray(cos, np.float32)[0, 0]              # (S, 64)
    sin2 = np.asarray(sin, np.float32)[0, 0]
    cosP = np.ascontiguousarray(
        np.concatenate([cos2.T, cos2.T], 0)).astype(bf16)  # [c; c]
    sinmP = np.ascontiguousarray(
        np.concatenate([-sin2.T, sin2.T], 0)).astype(bf16)  # [-s; s]
    kl = np.arange(128)[:, None]
    ql = np.arange(128)[None, :]
    triT = (kl <= ql).astype(bf16)
    eyeT = np.eye(128, dtype=np.float32).astype(bf16)
    ones_c = np.ones((128, 1), bf16)
    ones_r = np.ones((1, 128), np.float32)

    # wo reordered so row p = (j*8+r)*128 + d maps head (j*8+r), dim d
    j_ = np.arange(NREP)[:, None, None]
    r_ = np.arange(N_CORES)[None, :, None]
    d_ = np.arange(HD)[None, None, :]
    col_order = ((j_ * N_CORES + r_) * HD + d_).reshape(-1)
    woT_full = np.ascontiguousarray(
        np.asarray(wo, np.float32)[:, col_order].T)       # (4096 avrow, 4096)

    def pack_w(wT, ncols):  # (HID, ncols) -> (128, NHT*ncols), col h*ncols+c
        return np.ascontiguousarray(
            wT.reshape(NHT, 128, ncols).transpose(1, 0, 2)
            .reshape(128, NHT * ncols)).astype(bf16)

    in_maps = []
    for c in range(N_CORES):
        heads = [jj * N_CORES + c for jj in range(NREP)]
        wq_rows = np.concatenate([wq_s[h * HD:(h + 1) * HD, :] for h in heads],
                                 0)                        # (512, HID)
        wqP = pack_w(np.ascontiguousarray(wq_rows.T), 512)
        wkT_c = np.ascontiguousarray(
            np.asarray(wk, np.float32)[c * HD:(c + 1) * HD, :].T)
        wvT_c = np.ascontiguousarray(
            np.asarray(wv, np.float32)[c * HD:(c + 1) * HD, :].T)
        wkP = pack_w(wkT_c, 128)
        wvP = pack_w(wvT_c, 128)
        woP = pack_w(np.ascontiguousarray(
            woT_full[:, c * 512:(c + 1) * 512]), 512)
        in_maps.append(dict(xP=xP, wqP=wqP, wkP=wkP, wvP=wvP, woP=woP,
                            cosP=cosP, sinmP=sinmP, triT=triT, eyeT=eyeT,
                            ones_c=ones_c, ones_r=ones_r))
    return in_maps


def postprocess(results):
    out = np.empty((S, HID), np.float32)
    for c in range(N_CORES):
        out[:, c * 512:(c + 1) * 512] = results[c]["o_out"]
    return out[None]


def get_nc():
    if "nc" not in _CACHE:
        _CACHE["nc"] = build_nc()
    return _CACHE["nc"]


def kernel(hidden_states, wq, wk, wv, wo, cos, sin, causal_mask=None):
    nc = get_nc()
    in_maps = prep_inputs(hidden_states, wq, wk, wv, wo, cos, sin, causal_mask)
    res = run_bass_kernel_spmd(nc, in_maps, core_ids=list(range(N_CORES)))
    return postprocess(res.results)


# revision 21
# speedup vs baseline: 1.2686x; 1.0419x over previous
"""Tensor-parallel LlamaAttention (S=2048, HID=4096, NH=32, NKV=8) on 8 trn2 cores.

Sharding: core c owns q heads {c, c+8, c+16, c+24} (all share kv head c) and
kv head c.  Projections + attention fully local; per q-chunk C the normalized
avT slices of all 4 local heads are AllGathered (two half-gathers, j01/j23),
then each core computes its 512 o_proj output columns for that chunk's 4 seq
tiles (column-parallel wo) — so o_proj overlaps later attention chunks and
only the last chunk's gather is exposed.

Host packs all inputs partition-major so DMA lines are 8-32KB.
"""

import numpy as np
import ml_dtypes

import concourse.bacc as bacc
import concourse.tile as tile
import concourse.mybir as mybir
from concourse.bass_utils import run_bass_kernel_spmd

S = 2048
HID = 4096
NH = 32
NKV = 8
HD = 128
HALF = 64
N_CORES = 8
NREP = NH // NKV  # 4 q heads per core
NHT = HID // 128  # 32 hidden tiles
NST = S // 128    # 16 seq tiles
NSC = S // 512    # 4 seq chunks
BF16 = mybir.dt.bfloat16
F32 = mybir.dt.float32
F32R = mybir.dt.float32r

_CACHE = {}


def build_nc():
    nc = bacc.Bacc("TRN2", target_bir_lowering=False, debug=False,
                   num_devices=N_CORES)

    xP = nc.dram_tensor("xP", [128, NSC * NHT * 512], BF16,
                        kind="ExternalInput").ap()
    wqP = nc.dram_tensor("wqP", [128, NHT * 512], BF16,
                         kind="ExternalInput").ap()
    wkP = nc.dram_tensor("wkP", [128, NHT * 128], BF16,
                         kind="ExternalInput").ap()
    wvP = nc.dram_tensor("wvP", [128, NHT * 128], BF16,
                         kind="ExternalInput").ap()
    woP = nc.dram_tensor("woP", [128, NHT * 512], BF16,
                         kind="ExternalInput").ap()
    cosP = nc.dram_tensor("cosP", [HD, S], BF16, kind="ExternalInput").ap()
    sinmP = nc.dram_tensor("sinmP", [HD, S], BF16, kind="ExternalInput").ap()
    tri = nc.dram_tensor("triT", [128, 128], BF16, kind="ExternalInput").ap()
    eye = nc.dram_tensor("eyeT", [128, 128], BF16, kind="ExternalInput").ap()
    ones_c = nc.dram_tensor("ones_c", [128, 1], BF16, kind="ExternalInput").ap()
    ones_r = nc.dram_tensor("ones_r", [1, 128], F32, kind="ExternalInput").ap()

    o_out = nc.dram_tensor("o_out", [S, 512], F32, kind="ExternalOutput").ap()

    ag_in = [nc.dram_tensor(f"ag_in{c}", [128, 2048], BF16).ap()
             for c in range(NSC)]
    ag_out = [nc.dram_tensor(f"ag_out{c}", [N_CORES * 128, 2048], BF16,
                             addr_space="Shared").ap() for c in range(NSC)]

    with tile.TileContext(nc) as tc:
        _body(nc, tc, xP, wqP, wkP, wvP, woP, cosP, sinmP, tri, eye,
              ones_c, ones_r, o_out, ag_in, ag_out)
    nc.compile()
    return nc


def _body(nc, tc, xP, wqP, wkP, wvP, woP, cosP, sinmP, tri, eye,
          ones_c, ones_r, o_out, ag_in, ag_out):
    with tc.tile_pool(name="consts", bufs=1) as cpool:
        tri_sb = cpool.tile([128, 128], BF16, tag="tri")
        eye_sb = cpool.tile([128, 128], BF16, tag="eye")
        onc_sb = cpool.tile([128, 1], BF16, tag="onc")
        onr_sb = cpool.tile([1, 128], F32, tag="onr")
        nc.sync.dma_start(out=tri_sb[:], in_=tri[:])
        nc.sync.dma_start(out=eye_sb[:], in_=eye[:])
        nc.sync.dma_start(out=onc_sb[:], in_=ones_c[:])
        nc.sync.dma_start(out=onr_sb[:], in_=ones_r[:])

        with tc.tile_pool(name="qkv", bufs=1) as qkvpool:
            qT_sb = [qkvpool.tile([HD, S], BF16, tag=f"qT{j}", name=f"qT{j}")
                     for j in range(NREP)]
            kT_sb = qkvpool.tile([HD, S], BF16, tag="kT")
            v_sb = qkvpool.tile([128, S], BF16, tag="v")  # [s-in-tile, d]

            with (
                tc.tile_pool(name="rconsts", bufs=1) as rcpool,
                tc.tile_pool(name="wproj", bufs=1) as wpool,
                tc.tile_pool(name="xc", bufs=2) as xpool,
                tc.tile_pool(name="rope", bufs=2) as rpool,
                tc.tile_pool(name="pmm1", bufs=6, space="PSUM") as pmm1,
                tc.tile_pool(name="ptr", bufs=2, space="PSUM") as ptrp,
            ):
                _phase1(nc, tc, xP, wqP, wkP, wvP, cosP, sinmP, eye_sb,
                        qT_sb, kT_sb, v_sb, rcpool, wpool, xpool, rpool,
                        pmm1, ptrp)

            with (
                tc.tile_pool(name="wo", bufs=1) as wopool,
                tc.tile_pool(name="av", bufs=2) as avpool,
                tc.tile_pool(name="probs", bufs=16) as ppool,
                tc.tile_pool(name="srec", bufs=2) as srpool,
                tc.tile_pool(name="sbrd", bufs=2) as sbpool,
                tc.tile_pool(name="ag", bufs=16) as agpool,
                tc.tile_pool(name="oout", bufs=4) as opool,
                tc.tile_pool(name="pmm2", bufs=4, space="PSUM") as pmm2,
                tc.tile_pool(name="psav", bufs=2, space="PSUM") as psav,
                tc.tile_pool(name="psrs", bufs=2, space="PSUM") as psrs,
            ):
                _phase23(nc, tc, woP, o_out, ag_in, ag_out,
                         tri_sb, onc_sb, onr_sb,
                         qT_sb, kT_sb, v_sb, wopool, avpool, ppool,
                         srpool, sbpool, agpool, opool, pmm2, psav, psrs)


def _phase1(nc, tc, xP, wqP, wkP, wvP, cosP, sinmP, eye_sb,
            qT_sb, kT_sb, v_sb, rcpool, wpool, xpool, rpool, pmm1, ptrp):
    # one dma_start per tensor: its descriptors fan out across all 16 DMA
    # rings, and sync-engine issue cost stays minimal (the scalar engine's
    # queue stalls DMAs behind compute-dependent copies).
    cos_sb = rcpool.tile([HD, S], BF16, tag="cos")
    sinm_sb = rcpool.tile([HD, S], BF16, tag="sinm")
    nc.sync.dma_start(out=cos_sb[:], in_=cosP[:])
    nc.sync.dma_start(out=sinm_sb[:], in_=sinmP[:])

    wq_sb = wpool.tile([128, NHT * 512], BF16, tag="wq")
    wk_sb = wpool.tile([128, NHT * 128], BF16, tag="wk")
    wv_sb = wpool.tile([128, NHT * 128], BF16, tag="wv")
    nc.sync.dma_start(out=wq_sb[:], in_=wqP[:])
    nc.sync.dma_start(out=wk_sb[:], in_=wkP[:])
    nc.sync.dma_start(out=wv_sb[:], in_=wvP[:])

    def _rope(dst, pp, sc):
        # dst[:, sc] = rot(pp): [0:64] = x1*c - x2*s ; [64:128] = x2*c + x1*s
        # cos_sb = [c; c], sinm_sb = [-s; s] (signs baked host-side)
        pq_sb = rpool.tile([128, 512], BF16, tag="pq")
        nc.scalar.copy(pq_sb[:], pp[:])
        sw = rpool.tile([128, 512], BF16, tag="sw")
        nc.vector.tensor_scalar_mul(sw[0:HALF, :], pq_sb[HALF:128, :], 1.0)
        nc.vector.tensor_scalar_mul(sw[HALF:128, :], pq_sb[0:HALF, :], 1.0)
        a = rpool.tile([128, 512], BF16, tag="ra")
        b = rpool.tile([128, 512], BF16, tag="rb")
        nc.vector.tensor_mul(a[:], pq_sb[:], cos_sb[:, sc])
        nc.vector.tensor_mul(b[:], sw[:], sinm_sb[:, sc])
        nc.vector.tensor_add(dst[:, sc], a[:], b[:])

    for cs in range(NSC):
        sc = slice(cs * 512, (cs + 1) * 512)
        x_sb = xpool.tile([128, NHT * 512], BF16, tag="x", name=f"x{cs}")
        nc.sync.dma_start(out=x_sb[:],
                          in_=xP[:, cs * NHT * 512:(cs + 1) * NHT * 512])

        for j in range(NREP):
            pq = pmm1.tile([128, 512], F32, tag="mm", name=f"pq{cs}_{j}")
            for h in range(NHT):
                nc.tensor.matmul(
                    pq[:],
                    wq_sb[:, h * 512 + j * 128: h * 512 + (j + 1) * 128],
                    x_sb[:, h * 512:(h + 1) * 512],
                    start=(h == 0), stop=(h == NHT - 1))
            _rope(qT_sb[j], pq, sc)

        pk = pmm1.tile([128, 512], F32, tag="mm", name=f"pk{cs}")
        for h in range(NHT):
            nc.tensor.matmul(pk[:], wk_sb[:, h * 128:(h + 1) * 128],
                             x_sb[:, h * 512:(h + 1) * 512],
                             start=(h == 0), stop=(h == NHT - 1))
        _rope(kT_sb, pk, sc)

        # vT chunk [d, s'] with N=512 matmuls, then PE-transpose to [s, d]
        pv = pmm1.tile([128, 512], F32, tag="mm", name=f"pv{cs}")
        for h in range(NHT):
            nc.tensor.matmul(pv[:], wv_sb[:, h * 128:(h + 1) * 128],
                             x_sb[:, h * 512:(h + 1) * 512],
                             start=(h == 0), stop=(h == NHT - 1))
        vt_sb = rpool.tile([128, 512], BF16, tag="vt", name=f"vt{cs}")
        nc.scalar.copy(vt_sb[:], pv[:])
        for tl in range(4):
            pt_ps = ptrp.tile([128, 128], BF16, tag="tr", name=f"vtr{cs}_{tl}")
            nc.tensor.transpose(pt_ps[:], vt_sb[:, tl * 128:(tl + 1) * 128],
                                eye_sb[:])
            t0 = (cs * 4 + tl) * 128
            nc.scalar.copy(v_sb[:, t0:t0 + 128], pt_ps[:])


def _phase23(nc, tc, woP, o_out, ag_in, ag_out,
             tri_sb, onc_sb, onr_sb, qT_sb, kT_sb, v_sb, wopool, avpool,
             ppool, srpool, sbpool, agpool, opool, pmm2, psav, psrs):
    Exp = mybir.ActivationFunctionType.Exp
    DEPTH = 3
    RG = [list(range(N_CORES))]

    wo_sb = wopool.tile([128, NHT * 512], BF16, tag="wo")
    nc.sync.dma_start(out=wo_sb[:], in_=woP[:])

    av_tiles = [avpool.tile([128, 2048], BF16, tag="av", name=f"av{C}")
                for C in range(NSC)]

    state = {"recip": None, "norm": None}
    rrecs = {}

    def make_recip(j, C, prs):
        def f():
            rrec = srpool.tile([1, 512], F32, tag="rrec", name=f"rrec{j}_{C}")
            nc.vector.reciprocal_approx_fast(rrec[:], prs[:])
            rrecs[(j, C)] = rrec
        return f

    def make_norm(j, C, pav):
        def f():
            rrec = rrecs.pop((j, C))
            pb = pmm2.tile([128, 512], F32, tag="mm", name=f"pb{j}_{C}")
            nc.tensor.matmul(pb[:], onr_sb[:], rrec[:], start=True, stop=True)
            bsb = sbpool.tile([128, 512], F32, tag="bsb", name=f"bsb{j}_{C}")
            nc.vector.tensor_scalar_mul(bsb[:], pb[:], 1.0)
            nc.vector.tensor_mul(av_tiles[C][:, j * 512:(j + 1) * 512],
                                 pav[:], bsb[:])
            if j == 3:
                # one AllGather per chunk: collectives serialize on the
                # gpsimd queue, so fewer+bigger is faster
                nc.sync.dma_start(out=ag_in[C][:], in_=av_tiles[C][:])
                nc.gpsimd.collective_compute(
                    "AllGather", mybir.AluOpType.bypass, replica_groups=RG,
                    ins=[ag_in[C][:]], outs=[ag_out[C][:]])
        return f

    def run_carry(kind):
        if state[kind] is not None:
            state[kind]()
            state[kind] = None

    def attn_unit(j, C):
        qc0 = C * 512
        nkt = 4 * C + 4
        pav = psav.tile([128, 512], F32, tag="av", name=f"pav{j}_{C}")
        prs = psrs.tile([1, 512], F32, tag="rs", name=f"prs{j}_{C}")
        pend = []

        def drain_one():
            kt2, pt2 = pend.pop(0)
            nc.tensor.matmul(prs[:], onc_sb[:], pt2[:],
                             start=(kt2 == 0), stop=(kt2 == nkt - 1))
            nc.tensor.matmul(pav[:], v_sb[:, kt2 * 128:(kt2 + 1) * 128],
                             pt2[:], start=(kt2 == 0), stop=(kt2 == nkt - 1))

        for kt in range(nkt):
            off = max(0, (kt - 4 * C) * 128)
            ps = pmm2.tile([128, 512], F32, tag="mm", name=f"ps{j}_{C}_{kt}")
            nc.tensor.matmul(ps[:, off:512],
                             kT_sb[:, kt * 128:(kt + 1) * 128],
                             qT_sb[j][:, qc0 + off: qc0 + 512],
                             start=True, stop=True)
            pt = ppool.tile([128, 512], BF16, tag="pt", name=f"pt{j}_{C}_{kt}")
            if off > 0:
                nc.vector.memset(pt[:, 0:off], 0.0)
            nc.scalar.activation(pt[:, off:512], ps[:, off:512], Exp)
            if kt >= 4 * C:
                nc.vector.tensor_mul(pt[:, off:off + 128],
                                     pt[:, off:off + 128], tri_sb[:])
            pend.append((kt, pt))
            if kt == 1:
                run_carry("recip")
            if kt == 3:
                run_carry("norm")
            if len(pend) > DEPTH:
                drain_one()
        while pend:
            drain_one()
        state["recip"] = make_recip(j, C, prs)
        state["norm"] = make_norm(j, C, pav)

    def ph3(C):
        # readback 8 slabs [128, 2048]; slab r holds (j, r) tiles at
        # cols j*512 + q'; one sync dma_start each (16KB lines over rings)
        ts = []
        for r in range(N_CORES):
            t = agpool.tile([128, 2048], BF16, tag="ag", name=f"ag{C}_{r}")
            nc.sync.dma_start(out=t[:],
                              in_=ag_out[C][r * 128:(r + 1) * 128, :])
            ts.append(t)
        for stl in range(4):
            st = 4 * C + stl
            po = pmm2.tile([128, 512], F32, tag="mm", name=f"po{st}")
            n = 0
            for jgrp in range(NREP):
                for r in range(N_CORES):
                    nc.tensor.matmul(
                        po[:],
                        ts[r][:, jgrp * 512 + stl * 128:
                              jgrp * 512 + (stl + 1) * 128],
                        wo_sb[:, (jgrp * 8 + r) * 512:(jgrp * 8 + r + 1) * 512],
                        start=(n == 0), stop=(n == NREP * N_CORES - 1))
                    n += 1
            osb = opool.tile([128, 512], F32, tag="o", name=f"o{st}")
            nc.vector.tensor_scalar_mul(osb[:], po[:], 1.0)
            nc.sync.dma_start(out=o_out[st * 128:(st + 1) * 128, :],
                              in_=osb[:])

    # AG(C) fires in unit (C+1, 0)'s norm carry; the 4 AGs serialize on
    # gpsimd (~28us each), completing at roughly 44/72/100/128us into
    # phase 2 — consume each only after it has landed.
    PH3_AT = {(2, 2): 0, (3, 1): 1}
    for C in range(NSC):
        for j in range(NREP):
            attn_unit(j, C)
            if (C, j) in PH3_AT:
                ph3(PH3_AT[(C, j)])
    run_carry("recip")
    run_carry("norm")
    ph3(NSC - 2)
    ph3(NSC - 1)


def prep_inputs(hidden_states, wq, wk, wv, wo, cos, sin, causal_mask=None):
    bf16 = ml_dtypes.bfloat16
    x = np.asarray(hidden_states, np.float32)[0]          # (S, HID)
    xT = np.ascontiguousarray(x.T)                        # (HID, S)
    # chunk-major pack: xP[p, cs*16384 + h*512 + s'] = xT[h*128+p, cs*512+s']
    xP = np.ascontiguousarray(
        xT.reshape(NHT, 128, NSC, 512).transpose(1, 2, 0, 3)
        .reshape(128, NSC * NHT * 512)).astype(bf16)
    wq_s = np.asarray(wq, np.float32) / np.sqrt(HD)
    cos2 = np.asarray(cos, np.float32)[0, 0]              # (S, 64)
    sin2 = np.asarray(sin, np.float32)[0, 0]
    cosP = np.ascontiguousarray(
        np.concatenate([cos2.T, cos2.T], 0)).astype(bf16)  # [c; c]
    sinmP = np.ascontiguousarray(
        np.concatenate([-sin2.T, sin2.T], 0)).astype(bf16)  # [-s; s]
    kl = np.arange(128)[:, None]
    ql = np.arange(128)[None, :]
    triT = (kl <= ql).astype(bf16)
    eyeT = np.eye(128, dtype=np.float32).astype(bf16)
    ones_c = np.ones((128, 1), bf16)
    ones_r = np.ones((1, 128), np.float32)

    # wo reordered so row p = (j*8+r)*128 + d maps head (j*8+r), dim d
    j_ = np.arange(NREP)[:, None, None]
    r_ = np.arange(N_CORES)[None, :, None]
    d_ = np.arange(HD)[None, None, :]
    col_order = ((j_ * N_CORES + r_) * HD + d_).reshape(-1)
    woT_full = np.ascontiguousarray(
        np.asarray(wo, np.float32)[:, col_order].T)       # (4096 avrow, 4096)

    def pack_w(wT, ncols):  # (HID, ncols) -> (128, NHT*ncols), col h*ncols+c
        return np.ascontiguousarray(
            wT.reshape(NHT, 128, ncols).transpose(1, 0, 2)
            .reshape(128, NHT * ncols)).astype(bf16)

    in_maps = []
    for c in range(N_CORES):
        heads = [jj * N_CORES + c for jj in range(NREP)]
        wq_rows = np.concatenate([wq_s[h * HD:(h + 1) * HD, :] for h in heads],
                                 0)                        # (512, HID)
        wqP = pack_w(np.ascontiguousarray(wq_rows.T), 512)
        wkT_c = np.ascontiguousarray(
            np.asarray(wk, np.float32)[c * HD:(c + 1) * HD, :].T)
        wvT_c = np.ascontiguousarray(
            np.asarray(wv, np.float32)[c * HD:(c + 1) * HD, :].T)
        wkP = pack_w(wkT_c, 128)
        wvP = pack_w(wvT_c, 128)
        woP = pack_w(np.ascontiguousarray(
            woT_full[:, c * 512:(c + 1) * 512]), 512)
        in_maps.append(dict(xP=xP, wqP=wqP, wkP=wkP, wvP=wvP, woP=woP,
                            cosP=cosP, sinmP=sinmP, triT=triT, eyeT=eyeT,
                            ones_c=ones_c, ones_r=ones_r))
    return in_maps


def postprocess(results):
    out = np.empty((S, HID), np.float32)
    for c in range(N_CORES):
        out[:, c * 512:(c + 1) * 512] = results[c]["o_out"]
    return out[None]


def get_nc():
    if "nc" not in _CACHE:
        _CACHE["nc"] = build_nc()
    return _CACHE["nc"]


def kernel(hidden_states, wq, wk, wv, wo, cos, sin, causal_mask=None):
    nc = get_nc()
    in_maps = prep_inputs(hidden_states, wq, wk, wv, wo, cos, sin, causal_mask)
    res = run_bass_kernel_spmd(nc, in_maps, core_ids=list(range(N_CORES)))
    return postprocess(res.results)
